# revision 16
# baseline (speedup 1.0000x reference)
"""Trainium2 Bass kernel for gated multi-head attention (nn_MultiHeadAttention_57741540327756).

Reference computation (per batch b):
    q = x @ Wq.T, k = x @ Wk.T, v = x @ Wv.T     (split into h=16 heads of H=64)
    RoPE(q, k) with positions cumsum(mask)-1
    attn = softmax(causal+keymask(q k^T / sqrt(H)))
    out_h = attn @ v_h ; gated by sigmoid(x @ Wgate.T)[:, h]
    out = concat_h @ Wproj.T + bproj

Sharding: data-parallel over batch (2) x tensor-parallel over heads (4 heads/core).
Core c handles batch c//4, heads [4*(c%4), 4*(c%4)+4). Each core emits a partial
(T, D) projection output (bf16); host sums the 4 partials per batch in fp32 and
adds the bias.

Per-core dataflow (SPMD program, per-core data; all matmuls bf16 with fp32 PSUM):
  - Q^T,K^T computed transposed, (d, t) layout: lhsT = Wq.T chunk, rhs = x^T chunk.
  - RoPE during PSUM evacuation: t1 = q*cos (fp32), u = q*sin' (bf16, sin'
    row-swapped+signed on host), partition-swap of u via 4 SBUF->SBUF DMAs at
    full-T granularity, then q_rot = t1 + swap(u) on GPSIMD.
  - scores in (k, q) layout per head: lhsT = K^T (64, 128) slice; the two heads of
    a pair sit at base partitions 0/64 -> concurrent row-tiled matmuls.
  - exp on ScalarE over (128, <=1024) PSUM tiles with the 1/sqrt(H) scale folded
    in; causal handled by only computing k<=q blocks plus one 128x128 staircase
    multiply on diagonal blocks.
  - attnV: lhsT = V tile (128, 65) slice with a ones column -> PSUM row 64
    accumulates the softmax denominator L for free.
  - normalize/gate once per head pair over full T: f = sigmoid(gate)/L,
    gpsimd.partition_broadcast, one DVE multiply into G^T.
  - proj: lhsT = G^T tile slices, rhs = Wproj.T rows slice (bf16).
"""

import os
import sys

import numpy as np

for _p in ("/opt/trn_rl_repo", os.path.expanduser("~/.axon_site/_ro/trn_rl_repo")):
    if os.path.isdir(_p) and _p not in sys.path:
        sys.path.insert(0, _p)

import ml_dtypes  # noqa: E402

import concourse.bass as bass  # noqa: E402
import concourse.tile as tile  # noqa: E402
from concourse import bacc, mybir  # noqa: E402
from concourse.bass_utils import run_bass_kernel_spmd  # noqa: E402

# Problem constants (hardcoded per spec).
B, T, D, NH = 2, 2048, 1024, 16
H = D // NH  # 64 head dim
NCORES = 8
TPG = 4  # heads per core
DC = TPG * H  # 256 local dims per core
ROPE_BASE = 10000.0
SCALE = float(H) ** -0.5

F32 = mybir.dt.float32
BF16 = mybir.dt.bfloat16
BF = ml_dtypes.bfloat16

KQ = 128   # key chunk (psum partitions for scores)
QB = 1024  # query block for scores/exp
NCH = D // 128  # 8 contraction chunks

_PROGRAM_CACHE = {}


def _build_program():
    """Build the single-core SPMD Bass/Tile program."""
    nc = bacc.Bacc(
        "TRN2", target_bir_lowering=False, debug=False, num_devices=NCORES
    )

    aps = {}
    aps["xT"] = nc.dram_tensor("xT", [D, T], BF16, kind="ExternalInput").ap()
    aps["wqT"] = nc.dram_tensor("wqT", [D, DC], BF16, kind="ExternalInput").ap()
    aps["wkT"] = nc.dram_tensor("wkT", [D, DC], BF16, kind="ExternalInput").ap()
    aps["wvT"] = nc.dram_tensor("wvT", [D, TPG * 65], BF16, kind="ExternalInput").ap()
    aps["wgT"] = nc.dram_tensor("wgT", [128, NCH * TPG], BF16, kind="ExternalInput").ap()
    aps["wpT"] = nc.dram_tensor("wpT", [DC, D], BF16, kind="ExternalInput").ap()
    aps["cos2"] = nc.dram_tensor("cos2", [128, T], F32, kind="ExternalInput").ap()
    aps["sinSW"] = nc.dram_tensor("sinSW", [128, T], F32, kind="ExternalInput").ap()
    aps["stair"] = nc.dram_tensor("stair", [128, 128], BF16, kind="ExternalInput").ap()
    aps["out"] = nc.dram_tensor("out", [T, D], BF16, kind="ExternalOutput").ap()

    with tile.TileContext(nc) as tc:
        _emit(tc, aps)

    nc.compile()
    return nc


def _emit(tc, aps):
    nc = tc.nc
    from contextlib import ExitStack

    Exp = mybir.ActivationFunctionType.Exp
    Sigmoid = mybir.ActivationFunctionType.Sigmoid

    ctx = ExitStack()
    with ctx:
        # ---------------- pools ----------------
        xp = ctx.enter_context(tc.tile_pool(name="xp", bufs=8))       # x^T chunks
        wp = ctx.enter_context(tc.tile_pool(name="wp", bufs=1))       # weights/consts
        qk = ctx.enter_context(tc.tile_pool(name="qk", bufs=1))       # Q^T/K^T/G^T/V
        t1p = ctx.enter_context(tc.tile_pool(name="t1p", bufs=2))     # rope q*cos (f32)
        up = ctx.enter_context(tc.tile_pool(name="up", bufs=2))       # rope q*sin' (bf16)
        swp = ctx.enter_context(tc.tile_pool(name="swp", bufs=2))     # rope swapped
        pp = ctx.enter_context(tc.tile_pool(name="pp", bufs=4))       # P tiles (bf16)
        usp = ctx.enter_context(tc.tile_pool(name="usp", bufs=2))     # U staging (f32)
        rows = ctx.enter_context(tc.tile_pool(name="rows", bufs=1))   # L/f rows
        fbp = ctx.enter_context(tc.tile_pool(name="fbp", bufs=2))     # f broadcast
        gt = ctx.enter_context(tc.tile_pool(name="gtmp", bufs=2))     # odd-head G staging
        evp = ctx.enter_context(tc.tile_pool(name="evp", bufs=3))     # proj evac
        psS = ctx.enter_context(tc.tile_pool(name="psS", bufs=2, space="PSUM"))
        psQ = ctx.enter_context(tc.tile_pool(name="psQ", bufs=2, space="PSUM"))
        psO = ctx.enter_context(tc.tile_pool(name="psO", bufs=2, space="PSUM"))

        # ---------------- load inputs ----------------
        def wtile(key, c, cols, tag):
            t_ = wp.tile([128, cols], BF16, tag=f"{tag}{c}", name=f"{tag}{c}")
            nc.sync.dma_start(t_[:], aps[key][c * 128:(c + 1) * 128, :])
            return t_

        # gate weights first (tiny), then interleave wq/wk/x per contraction
        # chunk so the Q/K projection matmuls can start streaming immediately.
        wg_t = wp.tile([128, NCH * TPG], BF16, tag="wg")
        nc.sync.dma_start(wg_t[:], aps["wgT"][:, :])
        wq_t, wk_t, xts = [], [], []
        for c in range(NCH):
            wq_t.append(wtile("wqT", c, DC, "wq"))
            wk_t.append(wtile("wkT", c, DC, "wk"))
            xt = xp.tile([128, T], BF16, tag="x", name=f"x{c}")
            nc.sync.dma_start(xt[:], aps["xT"][c * 128:(c + 1) * 128, :])
            xts.append(xt)
        wv_t = [wtile("wvT", c, TPG * 65, "wv") for c in range(NCH)]
        cos_t = wp.tile([128, T], F32, tag="cos")
        nc.sync.dma_start(cos_t[:], aps["cos2"][:, :])
        sin_t = wp.tile([128, T], F32, tag="sin")
        nc.sync.dma_start(sin_t[:], aps["sinSW"][:, :])
        stair_t = wp.tile([128, 128], BF16, tag="stair")
        nc.sync.dma_start(stair_t[:], aps["stair"][:, :])
        wp_t = [wtile("wpT", c, D, "wpj") for c in range(DC // 128)]


        # ---------------- Q^T / K^T projections + RoPE ----------------
        # Mtile p holds heads (2p, 2p+1) stacked (64 rows each).
        QT = [qk.tile([128, T], BF16, tag=f"QT{p}", name=f"QT{p}") for p in range(2)]
        KT = [qk.tile([128, T], BF16, tag=f"KT{p}", name=f"KT{p}") for p in range(2)]
        GT = [qk.tile([128, T], BF16, tag=f"GT{p}", name=f"GT{p}") for p in range(2)]

        def emit_qk(p):
            for wts, dst in ((wq_t, QT[p]), (wk_t, KT[p])):
                t1 = t1p.tile([128, T], F32, tag="t1", name=f"t1_{p}")
                u = up.tile([128, T], BF16, tag="u", name=f"u_{p}")
                for j2 in range(T // 512):
                    sl2 = slice(j2 * 512, (j2 + 1) * 512)
                    ps = psQ.tile([128, 512], F32, tag="ps", name=f"qkps{p}{j2}")
                    for c in range(NCH):
                        nc.tensor.matmul(
                            ps[:],
                            wts[c][:, p * 128:(p + 1) * 128],
                            xts[c][:, sl2],
                            start=(c == 0),
                            stop=(c == NCH - 1),
                        )
                    nc.vector.tensor_mul(t1[:, sl2], ps[:], cos_t[:, sl2])
                    nc.vector.tensor_mul(u[:, sl2], ps[:], sin_t[:, sl2])
                # partition swap of u (32-row block exchange), full-T DMAs
                u2 = swp.tile([128, T], BF16, tag="sw", name=f"u2_{p}")
                for a, bb in ((0, 32), (32, 0), (64, 96), (96, 64)):
                    nc.sync.dma_start(u2[a:a + 32, :], u[bb:bb + 32, :])
                nc.gpsimd.tensor_add(dst[:], t1[:], u2[:])

        emit_qk(0)

        # ---------------- gate: gsig = sigmoid(x @ Wgate.T) as (4, T) -------------
        gsig = wp.tile([TPG, T], BF16, tag="gsig")
        for blk in range(T // 512):
            sl = slice(blk * 512, (blk + 1) * 512)
            gps = psQ.tile([TPG, 512], F32, tag="ps", name=f"gps{blk}")
            for c in range(NCH):
                nc.tensor.matmul(
                    gps[:], wg_t[:, 4 * c:4 * c + 4], xts[c][:, sl],
                    start=(c == 0), stop=(c == NCH - 1),
                )
            nc.scalar.activation(gps[:], gps[:], Sigmoid)
            nc.vector.tensor_copy(gsig[:, sl], gps[:])

        # ---------------- V projection (natural layout, ones col per head) --------
        # V tile per t-block: cols [65h .. 65h+63] = head h dims, col 65h+64 = 1.0
        V = []
        for tb in range(T // 128):
            vps = psQ.tile([128, TPG * 65], F32, tag="ps", name=f"vps{tb}")
            for c in range(NCH):
                nc.tensor.matmul(
                    vps[:],
                    xts[c][:, tb * 128:(tb + 1) * 128],
                    wv_t[c][:],
                    start=(c == 0),
                    stop=(c == NCH - 1),
                )
            nc.vector.memset(
                vps.rearrange("p (h c) -> p h c", c=65)[:, :, 64:65], 1.0
            )
            vt = qk.tile([128, TPG * 65], BF16, tag=f"V{tb}", name=f"V{tb}")
            nc.vector.tensor_copy(vt[:], vps[:])
            V.append(vt)

        emit_qk(1)

        # ---------------- attention per head pair ----------------
        # S2/P2 tiles pack both heads of the pair side by side ([A 512 | B 512])
        # so one exp instruction covers both heads.
        for p in range(2):
            us = [
                usp.tile([65, T], F32, tag="us", name=f"us{p}_{k}") for k in range(2)
            ]
            for j in range(T // 512):
                q0 = j * 512
                nk = (q0 + 512) // KQ
                oU = [
                    psO.tile([65, 512], F32, tag="oU", name=f"oU{p}_{j}_{k}")
                    for k in range(2)
                ]
                for i in range(nk):
                    k0 = i * KQ
                    off = max(0, k0 - q0)
                    diag = k0 >= q0
                    S2 = psS.tile([128, 1024], F32, tag="S2", name=f"S2_{p}_{j}_{i}")
                    for hh in range(2):  # head 2p+hh at rows [64hh:64hh+64]
                        r0, c0 = 64 * hh, 512 * hh
                        nc.tensor.matmul(
                            S2[:, c0 + off:c0 + 512],
                            KT[p][r0:r0 + 64, k0:k0 + KQ],
                            QT[p][r0:r0 + 64, q0 + off:q0 + 512],
                            start=True,
                            stop=True,
                        )
                    P2 = pp.tile([128, 1024], BF16, tag="P", name=f"P{p}_{j}_{i}")
                    if off == 0:
                        nc.scalar.activation(P2[:], S2[:], Exp, scale=SCALE)
                    else:
                        nc.scalar.activation(
                            P2.rearrange("p (h q) -> p h q", h=2)[:, :, off:],
                            S2.rearrange("p (h q) -> p h q", h=2)[:, :, off:],
                            Exp,
                            scale=SCALE,
                        )
                    if diag:
                        for hh in range(2):
                            c0 = 512 * hh
                            nc.vector.tensor_mul(
                                P2[:, c0 + off:c0 + off + 128],
                                P2[:, c0 + off:c0 + off + 128],
                                stair_t[:],
                            )
                    for hh in range(2):
                        h = 2 * p + hh
                        nc.tensor.matmul(
                            oU[hh][:, off:],
                            V[i][:, 65 * h:65 * h + 65],
                            P2[:, 512 * hh + off:512 * hh + 512],
                            start=(i == 0),
                            stop=(i == nk - 1),
                        )
                for hh in range(2):
                    nc.vector.tensor_copy(us[hh][:, q0:q0 + 512], oU[hh][:])
                if j % 2 == 1:
                    # normalize + gate + evacuate this 1024-block into G^T
                    jb = j // 2
                    qb0 = jb * QB
                    sl = slice(qb0, qb0 + QB)
                    l2 = rows.tile([2, QB], F32, tag="l2", name=f"l2_{p}_{jb}")
                    for hh in range(2):
                        nc.sync.dma_start(l2[hh:hh + 1, :], us[hh][64:65, sl])
                    g2 = rows.tile([2, QB], BF16, tag="g2", name=f"g2_{p}_{jb}")
                    nc.sync.dma_start(g2[:], gsig[2 * p:2 * p + 2, sl])
                    f2 = rows.tile([2, QB], F32, tag="f2", name=f"f2_{p}_{jb}")
                    nc.vector.reciprocal(f2[:], l2[:])
                    nc.vector.tensor_mul(f2[:], f2[:], g2[:])
                    for hh in range(2):
                        if hh == 0:
                            frow = f2[0:1, :]
                        else:
                            frow = rows.tile(
                                [1, QB], F32, tag="f1", name=f"f1_{p}_{jb}"
                            )
                            nc.sync.dma_start(frow[:], f2[1:2, :])
                        fb = fbp.tile([64, QB], F32, tag="fb", name=f"fb{p}_{jb}_{hh}")
                        nc.gpsimd.partition_broadcast(fb[:], frow[:])
                        if hh == 0:
                            nc.vector.tensor_mul(
                                GT[p][0:64, sl], us[hh][0:64, sl], fb[:]
                            )
                        else:
                            gtmp = gt.tile([64, QB], BF16, tag="gtmp")
                            nc.vector.tensor_mul(gtmp[:], us[hh][0:64, sl], fb[:])
                            nc.sync.dma_start(GT[p][64:128, sl], gtmp[:])

        # ---------------- output projection (partial; host adds bias+reduce) ------
        for tb in range(T // 128):
            t0 = tb * 128
            ev = evp.tile([128, D], BF16, tag="ev")
            for eb in range(D // 512):
                ops_ = psQ.tile([128, 512], F32, tag="ps", name=f"ops{tb}_{eb}")
                for c in range(2):
                    nc.tensor.matmul(
                        ops_[:],
                        GT[c][:, t0:t0 + 128],
                        wp_t[c][:, eb * 512:(eb + 1) * 512],
                        start=(c == 0),
                        stop=(c == 1),
                    )
                esl = slice(eb * 512, (eb + 1) * 512)
                if eb % 2 == 0:
                    nc.vector.tensor_copy(ev[:, esl], ops_[:])
                else:
                    nc.scalar.copy(ev[:, esl], ops_[:])
            nc.sync.dma_start(aps["out"][t0:t0 + 128, :], ev[:])


def _host_inputs(x, mask, Wq, Wk, Wv, Wgate, Wproj):
    """Build the 8 per-core input maps."""
    x = np.asarray(x, np.float32)
    mask = np.asarray(mask)
    pos = (np.cumsum(mask, axis=-1) - 1).astype(np.float32)  # (B, T)
    inv_freq = 1.0 / (ROPE_BASE ** (np.arange(0, H, 2, dtype=np.float32) / H))
    freqs = pos[:, :, None] * inv_freq[None, None, :]  # (B, T, 32)
    cos32 = np.cos(freqs)
    sin32 = np.sin(freqs)

    # cos tile (128, T): per 64-row head block, rows 0-31 and 32-63 share freqs.
    # sin' tile: [ +s, -s ] per head block (row-swapped signed sin), so that
    #   swap(q * sin') = rotate_half(q) * sin.
    def cos_tile(b):
        c = np.concatenate([cos32[b].T, cos32[b].T], axis=0)  # (64, T)
        return np.concatenate([c, c], axis=0).astype(np.float32)

    def sin_tile(b):
        s = sin32[b].T  # (32, T)
        blk = np.concatenate([s, -s], axis=0)  # (64, T)
        return np.concatenate([blk, blk], axis=0).astype(np.float32)

    stair = np.triu(np.ones((128, 128), np.float32)).astype(BF)

    WqT = np.ascontiguousarray(Wq.T.astype(np.float32))
    WkT = np.ascontiguousarray(Wk.T.astype(np.float32))
    WvT = np.ascontiguousarray(Wv.T.astype(np.float32))
    WgT = np.ascontiguousarray(Wgate.T.astype(np.float32))  # (D, h)
    WpT = np.ascontiguousarray(Wproj.T.astype(np.float32))

    in_maps = []
    for core in range(NCORES):
        b = core // TPG
        g = core % TPG
        sl = slice(g * DC, (g + 1) * DC)
        wv_loc = WvT[:, sl]
        wv_pad = np.zeros((D, TPG * 65), np.float32)
        for hl in range(TPG):
            wv_pad[:, 65 * hl:65 * hl + 64] = wv_loc[:, 64 * hl:64 * hl + 64]
        # pack gate weights (D, 4) -> (128, 8*4): cols 4c..4c+3 = chunk c
        wg_loc = WgT[:, g * TPG:(g + 1) * TPG]  # (D, 4)
        wg_pack = (
            wg_loc.reshape(NCH, 128, TPG).transpose(1, 0, 2).reshape(128, NCH * TPG)
        )
        in_maps.append(
            {
                "xT": np.ascontiguousarray(x[b].T).astype(BF),
                "wqT": np.ascontiguousarray(WqT[:, sl]).astype(BF),
                "wkT": np.ascontiguousarray(WkT[:, sl]).astype(BF),
                "wvT": wv_pad.astype(BF),
                "wgT": np.ascontiguousarray(wg_pack).astype(BF),
                "wpT": np.ascontiguousarray(WpT[sl, :]).astype(BF),
                "cos2": cos_tile(b),
                "sinSW": sin_tile(b),
                "stair": stair,
            }
        )
    return in_maps


def kernel(x, mask, Wq, Wk, Wv, Wgate, Wproj, bproj):
    x = np.asarray(x, np.float32)
    in_maps = _host_inputs(x, mask, Wq, Wk, Wv, Wgate, Wproj)

    if "nc" not in _PROGRAM_CACHE:
        _PROGRAM_CACHE["nc"] = _build_program()
    nc = _PROGRAM_CACHE["nc"]

    res = run_bass_kernel_spmd(nc, in_maps, core_ids=list(range(NCORES)))
    outs = [res.results[c]["out"] for c in range(NCORES)]

    bproj = np.asarray(bproj, np.float32)
    full = np.empty((B, T, D), np.float32)
    for b in range(B):
        acc = outs[TPG * b].astype(np.float32)
        for g in range(1, TPG):
            acc = acc + outs[TPG * b + g].astype(np.float32)
        full[b] = acc + bproj[None, :]
    return full


# revision 22
# speedup vs baseline: 33.7915x; 33.7915x over previous
"""Trainium2 Bass kernel for gated multi-head attention (nn_MultiHeadAttention_57741540327756).

Reference computation (per batch b):
    q = x @ Wq.T, k = x @ Wk.T, v = x @ Wv.T     (split into h=16 heads of H=64)
    RoPE(q, k) with positions cumsum(mask)-1
    attn = softmax(causal+keymask(q k^T / sqrt(H)))
    out_h = attn @ v_h ; gated by sigmoid(x @ Wgate.T)[:, h]
    out = concat_h @ Wproj.T + bproj

Sharding: data-parallel over batch (2) x tensor-parallel over heads (4 heads/core).
Core c handles batch c//4, heads [4*(c%4), 4*(c%4)+4). Each core emits a partial
(T, D) projection output (bf16); host sums the 4 partials per batch in fp32 and
adds the bias.

Per-core dataflow (SPMD program, per-core data; all matmuls bf16 with fp32 PSUM):
  - Q^T,K^T computed transposed, (d, t) layout: lhsT = Wq.T chunk, rhs = x^T chunk.
  - RoPE during PSUM evacuation: t1 = q*cos (fp32), u = q*sin' (bf16, sin'
    row-swapped+signed on host), partition-swap of u via 4 SBUF->SBUF DMAs at
    full-T granularity, then q_rot = t1 + swap(u) on GPSIMD.
  - scores in (k, q) layout per head: lhsT = K^T (64, 128) slice; the two heads of
    a pair sit at base partitions 0/64 -> concurrent row-tiled matmuls.
  - exp on ScalarE over (128, <=1024) PSUM tiles with the 1/sqrt(H) scale folded
    in; causal handled by only computing k<=q blocks plus one 128x128 staircase
    multiply on diagonal blocks.
  - attnV: lhsT = V tile (128, 65) slice with a ones column -> PSUM row 64
    accumulates the softmax denominator L for free.
  - normalize/gate once per head pair over full T: f = sigmoid(gate)/L,
    gpsimd.partition_broadcast, one DVE multiply into G^T.
  - proj: lhsT = G^T tile slices, rhs = Wproj.T rows slice (bf16).
"""

import os
import sys

import numpy as np

for _p in ("/opt/trn_rl_repo", os.path.expanduser("~/.axon_site/_ro/trn_rl_repo")):
    if os.path.isdir(_p) and _p not in sys.path:
        sys.path.insert(0, _p)

import ml_dtypes  # noqa: E402

import concourse.bass as bass  # noqa: E402
import concourse.tile as tile  # noqa: E402
from concourse import bacc, mybir  # noqa: E402
from concourse.bass_utils import run_bass_kernel_spmd  # noqa: E402

# Problem constants (hardcoded per spec).
B, T, D, NH = 2, 2048, 1024, 16
H = D // NH  # 64 head dim
NCORES = 8
TPG = 4  # heads per core
DC = TPG * H  # 256 local dims per core
ROPE_BASE = 10000.0
SCALE = float(H) ** -0.5

F32 = mybir.dt.float32
BF16 = mybir.dt.bfloat16
BF = ml_dtypes.bfloat16

KQ = 128   # key chunk (psum partitions for scores)
QB = 1024  # query block for scores/exp
NCH = D // 128  # 8 contraction chunks

_PROGRAM_CACHE = {}


def _build_program():
    """Build the single-core SPMD Bass/Tile program."""
    nc = bacc.Bacc(
        "TRN2", target_bir_lowering=False, debug=False, num_devices=NCORES
    )

    aps = {}
    aps["xT"] = nc.dram_tensor("xT", [D, T], BF16, kind="ExternalInput").ap()
    aps["wqT"] = nc.dram_tensor("wqT", [D, DC], BF16, kind="ExternalInput").ap()
    aps["wkT"] = nc.dram_tensor("wkT", [D, DC], BF16, kind="ExternalInput").ap()
    aps["wvT"] = nc.dram_tensor("wvT", [D, TPG * 65], BF16, kind="ExternalInput").ap()
    aps["wgT"] = nc.dram_tensor("wgT", [128, NCH * TPG], BF16, kind="ExternalInput").ap()
    aps["wpT"] = nc.dram_tensor("wpT", [DC, D], BF16, kind="ExternalInput").ap()
    aps["cos2"] = nc.dram_tensor("cos2", [128, T], F32, kind="ExternalInput").ap()
    aps["sinSW"] = nc.dram_tensor("sinSW", [128, T], F32, kind="ExternalInput").ap()
    aps["stair"] = nc.dram_tensor("stair", [128, 128], BF16, kind="ExternalInput").ap()
    aps["out"] = nc.dram_tensor("out", [T, D], BF16, kind="ExternalOutput").ap()

    with tile.TileContext(nc) as tc:
        _emit(tc, aps)

    nc.compile()
    return nc


def _emit(tc, aps):
    nc = tc.nc
    from contextlib import ExitStack

    Exp = mybir.ActivationFunctionType.Exp
    Sigmoid = mybir.ActivationFunctionType.Sigmoid

    ctx = ExitStack()
    with ctx:
        # ---------------- pools ----------------
        xp = ctx.enter_context(tc.tile_pool(name="xp", bufs=8))       # x^T chunks
        wp = ctx.enter_context(tc.tile_pool(name="wp", bufs=1))       # weights/consts
        qk = ctx.enter_context(tc.tile_pool(name="qk", bufs=1))       # Q^T/K^T/G^T/V
        t1p = ctx.enter_context(tc.tile_pool(name="t1p", bufs=2))     # rope q*cos (f32)
        up = ctx.enter_context(tc.tile_pool(name="up", bufs=2))       # rope q*sin' (bf16)
        swp = ctx.enter_context(tc.tile_pool(name="swp", bufs=2))     # rope swapped
        pp = ctx.enter_context(tc.tile_pool(name="pp", bufs=6))       # P tiles (bf16)
        usp = ctx.enter_context(tc.tile_pool(name="usp", bufs=2))     # U staging (f32)
        rows = ctx.enter_context(tc.tile_pool(name="rows", bufs=1))   # L/f rows
        fbp = ctx.enter_context(tc.tile_pool(name="fbp", bufs=2))     # f broadcast
        gt = ctx.enter_context(tc.tile_pool(name="gtmp", bufs=2))     # odd-head G staging
        evp = ctx.enter_context(tc.tile_pool(name="evp", bufs=3))     # proj evac
        psS = ctx.enter_context(tc.tile_pool(name="psS", bufs=2, space="PSUM"))
        psQ = ctx.enter_context(tc.tile_pool(name="psQ", bufs=2, space="PSUM"))
        psO = ctx.enter_context(tc.tile_pool(name="psO", bufs=2, space="PSUM"))

        # ---------------- load inputs ----------------
        def wtile(key, c, cols, tag):
            t_ = wp.tile([128, cols], BF16, tag=f"{tag}{c}", name=f"{tag}{c}")
            nc.sync.dma_start(t_[:], aps[key][c * 128:(c + 1) * 128, :])
            return t_

        # gate weights first (tiny), then interleave wq/wk/x per contraction
        # chunk so the Q/K projection matmuls can start streaming immediately.
        wg_t = wp.tile([128, NCH * TPG], BF16, tag="wg")
        nc.sync.dma_start(wg_t[:], aps["wgT"][:, :])
        wq_t, wk_t, xts = [], [], []
        for c in range(NCH):
            wq_t.append(wtile("wqT", c, DC, "wq"))
            wk_t.append(wtile("wkT", c, DC, "wk"))
            xt = xp.tile([128, T], BF16, tag="x", name=f"x{c}")
            nc.sync.dma_start(xt[:], aps["xT"][c * 128:(c + 1) * 128, :])
            xts.append(xt)
        wv_t = [wtile("wvT", c, TPG * 65, "wv") for c in range(NCH)]
        cos_t = wp.tile([128, T], F32, tag="cos")
        nc.sync.dma_start(cos_t[:], aps["cos2"][:, :])
        sin_t = wp.tile([128, T], F32, tag="sin")
        nc.sync.dma_start(sin_t[:], aps["sinSW"][:, :])
        stair_t = wp.tile([128, 128], BF16, tag="stair")
        nc.sync.dma_start(stair_t[:], aps["stair"][:, :])
        wp_t = [wtile("wpT", c, D, "wpj") for c in range(DC // 128)]


        # ---------------- Q^T / K^T projections + RoPE ----------------
        # Mtile p holds heads (2p, 2p+1) stacked (64 rows each).
        QT = [qk.tile([128, T], BF16, tag=f"QT{p}", name=f"QT{p}") for p in range(2)]
        KT = [qk.tile([128, T], BF16, tag=f"KT{p}", name=f"KT{p}") for p in range(2)]
        GT = [qk.tile([128, T], BF16, tag=f"GT{p}", name=f"GT{p}") for p in range(2)]

        def emit_qk(p):
            for wts, dst in ((wq_t, QT[p]), (wk_t, KT[p])):
                t1 = t1p.tile([128, T], F32, tag="t1", name=f"t1_{p}")
                u = up.tile([128, T], BF16, tag="u", name=f"u_{p}")
                for j2 in range(T // 512):
                    sl2 = slice(j2 * 512, (j2 + 1) * 512)
                    ps = psQ.tile([128, 512], F32, tag="ps", name=f"qkps{p}{j2}")
                    for c in range(NCH):
                        nc.tensor.matmul(
                            ps[:],
                            wts[c][:, p * 128:(p + 1) * 128],
                            xts[c][:, sl2],
                            start=(c == 0),
                            stop=(c == NCH - 1),
                        )
                    nc.vector.tensor_mul(t1[:, sl2], ps[:], cos_t[:, sl2])
                    nc.vector.tensor_mul(u[:, sl2], ps[:], sin_t[:, sl2])
                # partition swap of u (32-row block exchange), full-T DMAs
                u2 = swp.tile([128, T], BF16, tag="sw", name=f"u2_{p}")
                for a, bb in ((0, 32), (32, 0), (64, 96), (96, 64)):
                    nc.sync.dma_start(u2[a:a + 32, :], u[bb:bb + 32, :])
                nc.gpsimd.tensor_add(dst[:], t1[:], u2[:])

        emit_qk(0)

        # ---------------- gate: gsig = sigmoid(x @ Wgate.T) as (4, T) -------------
        gsig = wp.tile([TPG, T], BF16, tag="gsig")
        for blk in range(T // 512):
            sl = slice(blk * 512, (blk + 1) * 512)
            gps = psQ.tile([TPG, 512], F32, tag="ps", name=f"gps{blk}")
            for c in range(NCH):
                nc.tensor.matmul(
                    gps[:], wg_t[:, 4 * c:4 * c + 4], xts[c][:, sl],
                    start=(c == 0), stop=(c == NCH - 1),
                )
            nc.scalar.activation(gps[:], gps[:], Sigmoid)
            nc.vector.tensor_copy(gsig[:, sl], gps[:])

        # ---------------- V projection (natural layout, ones col per head) --------
        # V tile per t-block: cols [65h .. 65h+63] = head h dims, col 65h+64 = 1.0
        V = []
        for tb in range(T // 128):
            vps = psQ.tile([128, TPG * 65], F32, tag="ps", name=f"vps{tb}")
            for c in range(NCH):
                nc.tensor.matmul(
                    vps[:],
                    xts[c][:, tb * 128:(tb + 1) * 128],
                    wv_t[c][:],
                    start=(c == 0),
                    stop=(c == NCH - 1),
                )
            nc.vector.memset(
                vps.rearrange("p (h c) -> p h c", c=65)[:, :, 64:65], 1.0
            )
            vt = qk.tile([128, TPG * 65], BF16, tag=f"V{tb}", name=f"V{tb}")
            nc.vector.tensor_copy(vt[:], vps[:])
            V.append(vt)

        emit_qk(1)

        # ---------------- attention per head pair ----------------
        # S2/P2 tiles pack both heads of the pair side by side ([A 512 | B 512])
        # so one exp instruction covers both heads.
        for p in range(2):
            us = [
                usp.tile([65, T], F32, tag="us", name=f"us{p}_{k}") for k in range(2)
            ]
            for j in range(T // 512):
                q0 = j * 512
                nk = (q0 + 512) // KQ
                oU = [
                    psO.tile([65, 512], F32, tag="oU", name=f"oU{p}_{j}_{k}")
                    for k in range(2)
                ]
                for i in range(nk):
                    k0 = i * KQ
                    off = max(0, k0 - q0)
                    diag = k0 >= q0
                    S2 = psS.tile([128, 1024], F32, tag="S2", name=f"S2_{p}_{j}_{i}")
                    for hh in range(2):  # head 2p+hh at rows [64hh:64hh+64]
                        r0, c0 = 64 * hh, 512 * hh
                        nc.tensor.matmul(
                            S2[:, c0 + off:c0 + 512],
                            KT[p][r0:r0 + 64, k0:k0 + KQ],
                            QT[p][r0:r0 + 64, q0 + off:q0 + 512],
                            start=True,
                            stop=True,
                        )
                    P2 = pp.tile([128, 1024], BF16, tag="P", name=f"P{p}_{j}_{i}")
                    if off == 0:
                        nc.scalar.activation(P2[:], S2[:], Exp, scale=SCALE)
                    else:
                        nc.scalar.activation(
                            P2.rearrange("p (h q) -> p h q", h=2)[:, :, off:],
                            S2.rearrange("p (h q) -> p h q", h=2)[:, :, off:],
                            Exp,
                            scale=SCALE,
                        )
                    if diag:
                        for hh in range(2):
                            c0 = 512 * hh
                            nc.vector.tensor_mul(
                                P2[:, c0 + off:c0 + off + 128],
                                P2[:, c0 + off:c0 + off + 128],
                                stair_t[:],
                            )
                    for hh in range(2):
                        h = 2 * p + hh
                        nc.tensor.matmul(
                            oU[hh][:, off:],
                            V[i][:, 65 * h:65 * h + 65],
                            P2[:, 512 * hh + off:512 * hh + 512],
                            start=(i == 0),
                            stop=(i == nk - 1),
                        )
                for hh in range(2):
                    nc.vector.tensor_copy(us[hh][:, q0:q0 + 512], oU[hh][:])
                if j % 2 == 1:
                    # normalize + gate + evacuate this 1024-block into G^T
                    jb = j // 2
                    qb0 = jb * QB
                    sl = slice(qb0, qb0 + QB)
                    l2 = rows.tile([2, QB], F32, tag="l2", name=f"l2_{p}_{jb}")
                    for hh in range(2):
                        nc.sync.dma_start(l2[hh:hh + 1, :], us[hh][64:65, sl])
                    g2 = rows.tile([2, QB], BF16, tag="g2", name=f"g2_{p}_{jb}")
                    nc.sync.dma_start(g2[:], gsig[2 * p:2 * p + 2, sl])
                    f2 = rows.tile([2, QB], F32, tag="f2", name=f"f2_{p}_{jb}")
                    nc.vector.reciprocal(f2[:], l2[:])
                    nc.vector.tensor_mul(f2[:], f2[:], g2[:])
                    for hh in range(2):
                        if hh == 0:
                            frow = f2[0:1, :]
                        else:
                            frow = rows.tile(
                                [1, QB], F32, tag="f1", name=f"f1_{p}_{jb}"
                            )
                            nc.sync.dma_start(frow[:], f2[1:2, :])
                        fb = fbp.tile([64, QB], F32, tag="fb", name=f"fb{p}_{jb}_{hh}")
                        nc.gpsimd.partition_broadcast(fb[:], frow[:])
                        if hh == 0:
                            nc.vector.tensor_mul(
                                GT[p][0:64, sl], us[hh][0:64, sl], fb[:]
                            )
                        else:
                            gtmp = gt.tile([64, QB], BF16, tag="gtmp")
                            nc.vector.tensor_mul(gtmp[:], us[hh][0:64, sl], fb[:])
                            nc.sync.dma_start(GT[p][64:128, sl], gtmp[:])

        # ---------------- output projection (partial; host adds bias+reduce) ------
        for tb in range(T // 128):
            t0 = tb * 128
            ev = evp.tile([128, D], BF16, tag="ev")
            for eb in range(D // 512):
                ops_ = psQ.tile([128, 512], F32, tag="ps", name=f"ops{tb}_{eb}")
                for c in range(2):
                    nc.tensor.matmul(
                        ops_[:],
                        GT[c][:, t0:t0 + 128],
                        wp_t[c][:, eb * 512:(eb + 1) * 512],
                        start=(c == 0),
                        stop=(c == 1),
                    )
                esl = slice(eb * 512, (eb + 1) * 512)
                if eb % 2 == 0:
                    nc.vector.tensor_copy(ev[:, esl], ops_[:])
                else:
                    nc.scalar.copy(ev[:, esl], ops_[:])
            nc.sync.dma_start(aps["out"][t0:t0 + 128, :], ev[:])


def _host_inputs(x, mask, Wq, Wk, Wv, Wgate, Wproj):
    """Build the 8 per-core input maps."""
    x = np.asarray(x, np.float32)
    mask = np.asarray(mask)
    pos = (np.cumsum(mask, axis=-1) - 1).astype(np.float32)  # (B, T)
    inv_freq = 1.0 / (ROPE_BASE ** (np.arange(0, H, 2, dtype=np.float32) / H))
    freqs = pos[:, :, None] * inv_freq[None, None, :]  # (B, T, 32)
    cos32 = np.cos(freqs)
    sin32 = np.sin(freqs)

    # cos tile (128, T): per 64-row head block, rows 0-31 and 32-63 share freqs.
    # sin' tile: [ +s, -s ] per head block (row-swapped signed sin), so that
    #   swap(q * sin') = rotate_half(q) * sin.
    def cos_tile(b):
        c = np.concatenate([cos32[b].T, cos32[b].T], axis=0)  # (64, T)
        return np.concatenate([c, c], axis=0).astype(np.float32)

    def sin_tile(b):
        s = sin32[b].T  # (32, T)
        blk = np.concatenate([s, -s], axis=0)  # (64, T)
        return np.concatenate([blk, blk], axis=0).astype(np.float32)

    stair = np.triu(np.ones((128, 128), np.float32)).astype(BF)

    WqT = np.ascontiguousarray(Wq.T.astype(np.float32))
    WkT = np.ascontiguousarray(Wk.T.astype(np.float32))
    WvT = np.ascontiguousarray(Wv.T.astype(np.float32))
    WgT = np.ascontiguousarray(Wgate.T.astype(np.float32))  # (D, h)
    WpT = np.ascontiguousarray(Wproj.T.astype(np.float32))

    in_maps = []
    for core in range(NCORES):
        b = core // TPG
        g = core % TPG
        sl = slice(g * DC, (g + 1) * DC)
        wv_loc = WvT[:, sl]
        wv_pad = np.zeros((D, TPG * 65), np.float32)
        for hl in range(TPG):
            wv_pad[:, 65 * hl:65 * hl + 64] = wv_loc[:, 64 * hl:64 * hl + 64]
        # pack gate weights (D, 4) -> (128, 8*4): cols 4c..4c+3 = chunk c
        wg_loc = WgT[:, g * TPG:(g + 1) * TPG]  # (D, 4)
        wg_pack = (
            wg_loc.reshape(NCH, 128, TPG).transpose(1, 0, 2).reshape(128, NCH * TPG)
        )
        in_maps.append(
            {
                "xT": np.ascontiguousarray(x[b].T).astype(BF),
                "wqT": np.ascontiguousarray(WqT[:, sl]).astype(BF),
                "wkT": np.ascontiguousarray(WkT[:, sl]).astype(BF),
                "wvT": wv_pad.astype(BF),
                "wgT": np.ascontiguousarray(wg_pack).astype(BF),
                "wpT": np.ascontiguousarray(WpT[sl, :]).astype(BF),
                "cos2": cos_tile(b),
                "sinSW": sin_tile(b),
                "stair": stair,
            }
        )
    return in_maps


def kernel(x, mask, Wq, Wk, Wv, Wgate, Wproj, bproj):
    x = np.asarray(x, np.float32)
    in_maps = _host_inputs(x, mask, Wq, Wk, Wv, Wgate, Wproj)

    if "nc" not in _PROGRAM_CACHE:
        _PROGRAM_CACHE["nc"] = _build_program()
    nc = _PROGRAM_CACHE["nc"]

    res = run_bass_kernel_spmd(nc, in_maps, core_ids=list(range(NCORES)))
    outs = [res.results[c]["out"] for c in range(NCORES)]

    bproj = np.asarray(bproj, np.float32)
    full = np.empty((B, T, D), np.float32)
    for b in range(B):
        acc = outs[TPG * b].astype(np.float32)
        for g in range(1, TPG):
            acc = acc + outs[TPG * b + g].astype(np.float32)
        full[b] = acc + bproj[None, :]
    return full


# revision 28
# speedup vs baseline: 459.6742x; 13.6033x over previous
"""Trainium2 Bass kernel for gated multi-head attention (nn_MultiHeadAttention_57741540327756).

Reference computation (per batch b):
    q = x @ Wq.T, k = x @ Wk.T, v = x @ Wv.T     (split into h=16 heads of H=64)
    RoPE(q, k) with positions cumsum(mask)-1
    attn = softmax(causal+keymask(q k^T / sqrt(H)))
    out_h = attn @ v_h ; gated by sigmoid(x @ Wgate.T)[:, h]
    out = concat_h @ Wproj.T + bproj

Sharding: data-parallel over batch (2) x tensor-parallel over heads (4 heads/core).
Core c handles batch c//4, heads [4*(c%4), 4*(c%4)+4). Each core emits a partial
(T, D) projection output (bf16); host sums the 4 partials per batch in fp32 and
adds the bias.

Per-core dataflow (SPMD program, per-core data; all matmuls bf16 with fp32 PSUM):
  - Q^T,K^T computed transposed, (d, t) layout: lhsT = Wq.T chunk, rhs = x^T chunk.
  - RoPE during PSUM evacuation: t1 = q*cos (fp32), u = q*sin' (bf16, sin'
    row-swapped+signed on host), partition-swap of u via 4 SBUF->SBUF DMAs at
    full-T granularity, then q_rot = t1 + swap(u) on GPSIMD.
  - scores in (k, q) layout per head: lhsT = K^T (64, 128) slice; the two heads of
    a pair sit at base partitions 0/64 -> concurrent row-tiled matmuls.
  - exp on ScalarE over (128, <=1024) PSUM tiles with the 1/sqrt(H) scale folded
    in; causal handled by only computing k<=q blocks plus one 128x128 staircase
    multiply on diagonal blocks.
  - attnV: lhsT = V tile (128, 65) slice with a ones column -> PSUM row 64
    accumulates the softmax denominator L for free.
  - normalize/gate once per head pair over full T: f = sigmoid(gate)/L,
    gpsimd.partition_broadcast, one DVE multiply into G^T.
  - proj: lhsT = G^T tile slices, rhs = Wproj.T rows slice (bf16).
"""

import os
import sys

import numpy as np

for _p in ("/opt/trn_rl_repo", os.path.expanduser("~/.axon_site/_ro/trn_rl_repo")):
    if os.path.isdir(_p) and _p not in sys.path:
        sys.path.insert(0, _p)

import ml_dtypes  # noqa: E402

import concourse.bass as bass  # noqa: E402
import concourse.tile as tile  # noqa: E402
from concourse import bacc, mybir  # noqa: E402
from concourse.bass_utils import run_bass_kernel_spmd  # noqa: E402

# Problem constants (hardcoded per spec).
B, T, D, NH = 2, 2048, 1024, 16
H = D // NH  # 64 head dim
NCORES = 8
TPG = 4  # heads per core
DC = TPG * H  # 256 local dims per core
ROPE_BASE = 10000.0
SCALE = float(H) ** -0.5

F32 = mybir.dt.float32
BF16 = mybir.dt.bfloat16
BF = ml_dtypes.bfloat16

KQ = 128   # key chunk (psum partitions for scores)
QB = 1024  # query block for scores/exp
NCH = D // 128  # 8 contraction chunks

_PROGRAM_CACHE = {}


def _build_program():
    """Build the single-core SPMD Bass/Tile program."""
    nc = bacc.Bacc(
        "TRN2", target_bir_lowering=False, debug=False, num_devices=NCORES
    )

    aps = {}
    aps["xT"] = nc.dram_tensor("xT", [D, T], BF16, kind="ExternalInput").ap()
    aps["wqT"] = nc.dram_tensor("wqT", [D, DC], BF16, kind="ExternalInput").ap()
    aps["wkT"] = nc.dram_tensor("wkT", [D, DC], BF16, kind="ExternalInput").ap()
    aps["wvT"] = nc.dram_tensor("wvT", [D, TPG * 65], BF16, kind="ExternalInput").ap()
    aps["wgT"] = nc.dram_tensor("wgT", [128, NCH * TPG], BF16, kind="ExternalInput").ap()
    aps["wpT"] = nc.dram_tensor("wpT", [DC, D], BF16, kind="ExternalInput").ap()
    aps["cos2"] = nc.dram_tensor("cos2", [128, T], F32, kind="ExternalInput").ap()
    aps["sinSW"] = nc.dram_tensor("sinSW", [128, T], F32, kind="ExternalInput").ap()
    aps["stair"] = nc.dram_tensor("stair", [128, 128], BF16, kind="ExternalInput").ap()
    aps["out"] = nc.dram_tensor("out", [T, D], BF16, kind="ExternalOutput").ap()

    with tile.TileContext(nc) as tc:
        _emit(tc, aps)

    nc.compile()
    return nc


def _emit(tc, aps):
    nc = tc.nc
    from contextlib import ExitStack

    Exp = mybir.ActivationFunctionType.Exp
    Sigmoid = mybir.ActivationFunctionType.Sigmoid

    ctx = ExitStack()
    with ctx:
        # ---------------- pools ----------------
        xp = ctx.enter_context(tc.tile_pool(name="xp", bufs=8))       # x^T chunks
        wp = ctx.enter_context(tc.tile_pool(name="wp", bufs=1))       # weights/consts
        qk = ctx.enter_context(tc.tile_pool(name="qk", bufs=1))       # Q^T/K^T/G^T/V
        t1p = ctx.enter_context(tc.tile_pool(name="t1p", bufs=2))     # rope q*cos (f32)
        up = ctx.enter_context(tc.tile_pool(name="up", bufs=2))       # rope q*sin' (bf16)
        swp = ctx.enter_context(tc.tile_pool(name="swp", bufs=2))     # rope swapped
        pp = ctx.enter_context(tc.tile_pool(name="pp", bufs=8))       # P tiles (bf16)
        usp = ctx.enter_context(tc.tile_pool(name="usp", bufs=2))     # U staging (f32)
        rows = ctx.enter_context(tc.tile_pool(name="rows", bufs=2))   # L/f rows
        fbp = ctx.enter_context(tc.tile_pool(name="fbp", bufs=2))     # f broadcast
        gt = ctx.enter_context(tc.tile_pool(name="gtmp", bufs=2))     # odd-head G staging
        evp = ctx.enter_context(tc.tile_pool(name="evp", bufs=3))     # proj evac
        psS = ctx.enter_context(tc.tile_pool(name="psS", bufs=2, space="PSUM"))
        psQ = ctx.enter_context(tc.tile_pool(name="psQ", bufs=2, space="PSUM"))
        psO = ctx.enter_context(tc.tile_pool(name="psO", bufs=2, space="PSUM"))

        # ---------------- load inputs ----------------
        def wtile(key, c, cols, tag):
            t_ = wp.tile([128, cols], BF16, tag=f"{tag}{c}", name=f"{tag}{c}")
            nc.sync.dma_start(t_[:], aps[key][c * 128:(c + 1) * 128, :])
            return t_

        # gate weights first (tiny), then interleave wq/wk/x per contraction
        # chunk so the Q/K projection matmuls can start streaming immediately.
        wg_t = wp.tile([128, NCH * TPG], BF16, tag="wg")
        nc.sync.dma_start(wg_t[:], aps["wgT"][:, :])
        wq_t, wk_t, xts = [], [], []
        for c in range(NCH):
            wq_t.append(wtile("wqT", c, DC, "wq"))
            wk_t.append(wtile("wkT", c, DC, "wk"))
            xt = xp.tile([128, T], BF16, tag="x", name=f"x{c}")
            nc.sync.dma_start(xt[:], aps["xT"][c * 128:(c + 1) * 128, :])
            xts.append(xt)
        wv_t = [wtile("wvT", c, TPG * 65, "wv") for c in range(NCH)]
        cos_t = wp.tile([128, T], F32, tag="cos")
        nc.sync.dma_start(cos_t[:], aps["cos2"][:, :])
        sin_t = wp.tile([128, T], F32, tag="sin")
        nc.sync.dma_start(sin_t[:], aps["sinSW"][:, :])
        stair_t = wp.tile([128, 128], BF16, tag="stair")
        nc.sync.dma_start(stair_t[:], aps["stair"][:, :])
        wp_t = [wtile("wpT", c, D, "wpj") for c in range(DC // 128)]


        # ---------------- Q^T / K^T projections + RoPE ----------------
        # Mtile p holds heads (2p, 2p+1) stacked (64 rows each).
        QT = [qk.tile([128, T], BF16, tag=f"QT{p}", name=f"QT{p}") for p in range(2)]
        KT = [qk.tile([128, T], BF16, tag=f"KT{p}", name=f"KT{p}") for p in range(2)]
        GT = [qk.tile([128, T], BF16, tag=f"GT{p}", name=f"GT{p}") for p in range(2)]

        def emit_qk(p):
            for wts, dst in ((wq_t, QT[p]), (wk_t, KT[p])):
                t1 = t1p.tile([128, T], F32, tag="t1", name=f"t1_{p}")
                u = up.tile([128, T], BF16, tag="u", name=f"u_{p}")
                for j2 in range(T // 512):
                    sl2 = slice(j2 * 512, (j2 + 1) * 512)
                    ps = psQ.tile([128, 512], F32, tag="ps", name=f"qkps{p}{j2}")
                    for c in range(NCH):
                        nc.tensor.matmul(
                            ps[:],
                            wts[c][:, p * 128:(p + 1) * 128],
                            xts[c][:, sl2],
                            start=(c == 0),
                            stop=(c == NCH - 1),
                        )
                    nc.vector.tensor_mul(t1[:, sl2], ps[:], cos_t[:, sl2])
                    nc.vector.tensor_mul(u[:, sl2], ps[:], sin_t[:, sl2])
                # partition swap of u (32-row block exchange), full-T DMAs
                u2 = swp.tile([128, T], BF16, tag="sw", name=f"u2_{p}")
                for a, bb in ((0, 32), (32, 0), (64, 96), (96, 64)):
                    nc.sync.dma_start(u2[a:a + 32, :], u[bb:bb + 32, :])
                nc.gpsimd.tensor_add(dst[:], t1[:], u2[:])

        emit_qk(0)

        # ---------------- gate: gsig = sigmoid(x @ Wgate.T) as (4, T) -------------
        gsig = wp.tile([TPG, T], BF16, tag="gsig")
        for blk in range(T // 512):
            sl = slice(blk * 512, (blk + 1) * 512)
            gps = psQ.tile([TPG, 512], F32, tag="ps", name=f"gps{blk}")
            for c in range(NCH):
                nc.tensor.matmul(
                    gps[:], wg_t[:, 4 * c:4 * c + 4], xts[c][:, sl],
                    start=(c == 0), stop=(c == NCH - 1),
                )
            nc.scalar.activation(gps[:], gps[:], Sigmoid)
            nc.vector.tensor_copy(gsig[:, sl], gps[:])

        # ---------------- V projection (natural layout, ones col per head) --------
        # V tile per t-block: cols [65h .. 65h+63] = head h dims, col 65h+64 = 1.0
        V = []
        for tb in range(T // 128):
            vps = psQ.tile([128, TPG * 65], F32, tag="ps", name=f"vps{tb}")
            for c in range(NCH):
                nc.tensor.matmul(
                    vps[:],
                    xts[c][:, tb * 128:(tb + 1) * 128],
                    wv_t[c][:],
                    start=(c == 0),
                    stop=(c == NCH - 1),
                )
            nc.vector.memset(
                vps.rearrange("p (h c) -> p h c", c=65)[:, :, 64:65], 1.0
            )
            vt = qk.tile([128, TPG * 65], BF16, tag=f"V{tb}", name=f"V{tb}")
            nc.vector.tensor_copy(vt[:], vps[:])
            V.append(vt)

        emit_qk(1)

        # ---------------- attention per head pair ----------------
        # S2/P2 tiles pack both heads of the pair side by side ([A 512 | B 512])
        # so one exp instruction covers both heads.
        for p in range(2):
            us = [
                usp.tile([65, T], F32, tag="us", name=f"us{p}_{k}") for k in range(2)
            ]
            for j in range(T // 512):
                q0 = j * 512
                nk = (q0 + 512) // KQ
                oU = [
                    psO.tile([65, 512], F32, tag="oU", name=f"oU{p}_{j}_{k}")
                    for k in range(2)
                ]
                for i in range(nk):
                    k0 = i * KQ
                    off = max(0, k0 - q0)
                    diag = k0 >= q0
                    S2 = psS.tile([128, 1024], F32, tag="S2", name=f"S2_{p}_{j}_{i}")
                    for hh in range(2):  # head 2p+hh at rows [64hh:64hh+64]
                        r0, c0 = 64 * hh, 512 * hh
                        nc.tensor.matmul(
                            S2[:, c0 + off:c0 + 512],
                            KT[p][r0:r0 + 64, k0:k0 + KQ],
                            QT[p][r0:r0 + 64, q0 + off:q0 + 512],
                            start=True,
                            stop=True,
                        )
                    P2 = pp.tile([128, 1024], BF16, tag="P", name=f"P{p}_{j}_{i}")
                    if off == 0:
                        nc.scalar.activation(P2[:], S2[:], Exp, scale=SCALE)
                    else:
                        nc.scalar.activation(
                            P2.rearrange("p (h q) -> p h q", h=2)[:, :, off:],
                            S2.rearrange("p (h q) -> p h q", h=2)[:, :, off:],
                            Exp,
                            scale=SCALE,
                        )
                    if diag:
                        for hh in range(2):
                            c0 = 512 * hh
                            nc.vector.tensor_mul(
                                P2[:, c0 + off:c0 + off + 128],
                                P2[:, c0 + off:c0 + off + 128],
                                stair_t[:],
                            )
                    for hh in range(2):
                        h = 2 * p + hh
                        nc.tensor.matmul(
                            oU[hh][:, off:],
                            V[i][:, 65 * h:65 * h + 65],
                            P2[:, 512 * hh + off:512 * hh + 512],
                            start=(i == 0),
                            stop=(i == nk - 1),
                        )
                for hh in range(2):
                    nc.vector.tensor_copy(us[hh][:, q0:q0 + 512], oU[hh][:])
                # normalize + gate + evacuate this 512-block into G^T
                sl = slice(q0, q0 + 512)
                l2 = rows.tile([2, 512], F32, tag="l2", name=f"l2_{p}_{j}")
                for hh in range(2):
                    nc.sync.dma_start(l2[hh:hh + 1, :], us[hh][64:65, sl])
                g2 = rows.tile([2, 512], BF16, tag="g2", name=f"g2_{p}_{j}")
                nc.sync.dma_start(g2[:], gsig[2 * p:2 * p + 2, sl])
                f2 = rows.tile([2, 512], F32, tag="f2", name=f"f2_{p}_{j}")
                nc.vector.reciprocal(f2[:], l2[:])
                nc.vector.tensor_mul(f2[:], f2[:], g2[:])
                for hh in range(2):
                    if hh == 0:
                        frow = f2[0:1, :]
                    else:
                        frow = rows.tile(
                            [1, 512], F32, tag="f1", name=f"f1_{p}_{j}"
                        )
                        nc.sync.dma_start(frow[:], f2[1:2, :])
                    fb = fbp.tile([64, 512], F32, tag="fb", name=f"fb{p}_{j}_{hh}")
                    nc.gpsimd.partition_broadcast(fb[:], frow[:])
                    if hh == 0:
                        nc.vector.tensor_mul(
                            GT[p][0:64, sl], us[hh][0:64, sl], fb[:]
                        )
                    else:
                        gtmp = gt.tile([64, 512], BF16, tag="gtmp")
                        nc.vector.tensor_mul(gtmp[:], us[hh][0:64, sl], fb[:])
                        nc.sync.dma_start(GT[p][64:128, sl], gtmp[:])

        # ---------------- output projection (partial; host adds bias+reduce) ------
        for tb in range(T // 128):
            t0 = tb * 128
            ev = evp.tile([128, D], BF16, tag="ev")
            for eb in range(D // 512):
                pool_ = psQ if (tb + eb) % 2 == 0 else psO
                tag_ = "ps" if (tb + eb) % 2 == 0 else "oU"
                ops_ = pool_.tile([128, 512], F32, tag=tag_, name=f"ops{tb}_{eb}")
                for c in range(2):
                    nc.tensor.matmul(
                        ops_[:],
                        GT[c][:, t0:t0 + 128],
                        wp_t[c][:, eb * 512:(eb + 1) * 512],
                        start=(c == 0),
                        stop=(c == 1),
                    )
                esl = slice(eb * 512, (eb + 1) * 512)
                if eb % 2 == 0:
                    nc.vector.tensor_copy(ev[:, esl], ops_[:])
                else:
                    nc.scalar.copy(ev[:, esl], ops_[:])
            nc.sync.dma_start(aps["out"][t0:t0 + 128, :], ev[:])


def _host_inputs(x, mask, Wq, Wk, Wv, Wgate, Wproj):
    """Build the 8 per-core input maps."""
    x = np.asarray(x, np.float32)
    mask = np.asarray(mask)
    pos = (np.cumsum(mask, axis=-1) - 1).astype(np.float32)  # (B, T)
    inv_freq = 1.0 / (ROPE_BASE ** (np.arange(0, H, 2, dtype=np.float32) / H))
    freqs = pos[:, :, None] * inv_freq[None, None, :]  # (B, T, 32)
    cos32 = np.cos(freqs)
    sin32 = np.sin(freqs)

    # cos tile (128, T): per 64-row head block, rows 0-31 and 32-63 share freqs.
    # sin' tile: [ +s, -s ] per head block (row-swapped signed sin), so that
    #   swap(q * sin') = rotate_half(q) * sin.
    def cos_tile(b):
        c = np.concatenate([cos32[b].T, cos32[b].T], axis=0)  # (64, T)
        return np.concatenate([c, c], axis=0).astype(np.float32)

    def sin_tile(b):
        s = sin32[b].T  # (32, T)
        blk = np.concatenate([s, -s], axis=0)  # (64, T)
        return np.concatenate([blk, blk], axis=0).astype(np.float32)

    stair = np.triu(np.ones((128, 128), np.float32)).astype(BF)

    WqT = np.ascontiguousarray(Wq.T.astype(np.float32))
    WkT = np.ascontiguousarray(Wk.T.astype(np.float32))
    WvT = np.ascontiguousarray(Wv.T.astype(np.float32))
    WgT = np.ascontiguousarray(Wgate.T.astype(np.float32))  # (D, h)
    WpT = np.ascontiguousarray(Wproj.T.astype(np.float32))

    in_maps = []
    for core in range(NCORES):
        b = core // TPG
        g = core % TPG
        sl = slice(g * DC, (g + 1) * DC)
        wv_loc = WvT[:, sl]
        wv_pad = np.zeros((D, TPG * 65), np.float32)
        for hl in range(TPG):
            wv_pad[:, 65 * hl:65 * hl + 64] = wv_loc[:, 64 * hl:64 * hl + 64]
        # pack gate weights (D, 4) -> (128, 8*4): cols 4c..4c+3 = chunk c
        wg_loc = WgT[:, g * TPG:(g + 1) * TPG]  # (D, 4)
        wg_pack = (
            wg_loc.reshape(NCH, 128, TPG).transpose(1, 0, 2).reshape(128, NCH * TPG)
        )
        in_maps.append(
            {
                "xT": np.ascontiguousarray(x[b].T).astype(BF),
                "wqT": np.ascontiguousarray(WqT[:, sl]).astype(BF),
                "wkT": np.ascontiguousarray(WkT[:, sl]).astype(BF),
                "wvT": wv_pad.astype(BF),
                "wgT": np.ascontiguousarray(wg_pack).astype(BF),
                "wpT": np.ascontiguousarray(WpT[sl, :]).astype(BF),
                "cos2": cos_tile(b),
                "sinSW": sin_tile(b),
                "stair": stair,
            }
        )
    return in_maps


def kernel(x, mask, Wq, Wk, Wv, Wgate, Wproj, bproj):
    x = np.asarray(x, np.float32)
    in_maps = _host_inputs(x, mask, Wq, Wk, Wv, Wgate, Wproj)

    if "nc" not in _PROGRAM_CACHE:
        _PROGRAM_CACHE["nc"] = _build_program()
    nc = _PROGRAM_CACHE["nc"]

    res = run_bass_kernel_spmd(nc, in_maps, core_ids=list(range(NCORES)))
    outs = [res.results[c]["out"] for c in range(NCORES)]

    bproj = np.asarray(bproj, np.float32)
    full = np.empty((B, T, D), np.float32)
    for b in range(B):
        acc = outs[TPG * b].astype(np.float32)
        for g in range(1, TPG):
            acc = acc + outs[TPG * b + g].astype(np.float32)
        full[b] = acc + bproj[None, :]
    return full
